# revision 18
# baseline (speedup 1.0000x reference)
"""Trainium2 Bass kernel for nn_Decoder_21595095564698.

Gamma design
------------
Pure data parallel over batch (16384 -> 8 cores x 2048). Per core, batch is
laid out banded: 4 chunks of 512 on partition bands [32c, 32c+30).

* Phase A: the 48-layer encoder reduction streams over 2 HWDGE queues and is
  accumulated ON THE PE via float32r identity matmuls into PSUM (1 cyc/row),
  leaving vector/scalar/gpsimd idle. V4/Vg4 then come from PE transposes and
  a fused [61,60] map as before.
* Recurrence: every small GEMM is ONE 512-col matmul with a block-diagonal
  stationary matrix (blocks of <=32 rows per band). The LSTM pre-activation
  accumulates 5 such matmuls per gate bank ({y,bias}, V, z2-feedback, gh, h).
  The out->rin feedback is folded through z2 (rank-1 factorization
  W_ih[:,3] (x) Wf3), so no per-step SBUF stack writes exist at all.
* gate(s+1), d = h-V and gh = g*(h-V) are computed at the tail of step s,
  off the critical path of step s+1.
* out(s) collapses to a single [4,512] matmul (chunk-contiguous lhsT) plus
  one ACT extract into a [4, 12*512] store, DMA'd to DRAM at the end.
"""

import numpy as np

B, T, DIN, ENC_LEN, ENC, DEC = 16384, 13, 4, 48, 60, 30
NCORES = 8
BC = B // NCORES          # 2048 batch per core
NB = 4                    # chunks per core (partition bands)
ROW_MAP = [0, 30, 90, 60]  # LSTM gate banks: i, f, o, g(cell) -> W row offsets

_PROG = {}


def _local_of_dev():
    # banded position 512c + 256h + 128jp + p  <->  dev index in the enc shard
    idx = np.arange(BC)
    c, r = idx // 512, idx % 512
    h, r2 = r // 256, r % 256
    jp, p = r2 // 128, r2 % 128
    return 1024 * h + 8 * p + 4 * jp + c


def _build_program(repeat=1, steps=12):
    import sys
    if '/opt/trn_rl_repo' not in sys.path:
        sys.path.insert(0, '/opt/trn_rl_repo')
    import concourse.bass as bass  # noqa
    import concourse.tile as tile
    from concourse import bacc, mybir
    from concourse.masks import make_identity

    F32 = mybir.dt.float32
    F32R = mybir.dt.float32r
    F16 = mybir.dt.float16
    AF = mybir.ActivationFunctionType
    OP = mybir.AluOpType

    nc = bacc.Bacc("TRN2", target_bir_lowering=False, debug=False,
                   num_devices=NCORES)

    # ---- I/O ----
    enc_d = nc.dram_tensor("enc", [ENC_LEN, BC, ENC], F32R, kind="ExternalInput").ap()
    idr_d = nc.dram_tensor("identr", [128, 128], F32R, kind="ExternalInput").ap()
    ybd_d = nc.dram_tensor("ybd", [128, 6144], F16, kind="ExternalInput").ap()
    h0_d = nc.dram_tensor("h0", [128, 512], F16, kind="ExternalInput").ap()
    wkv_d = nc.dram_tensor("wkv61", [61, 60], F16, kind="ExternalInput").ap()
    wy0_d = nc.dram_tensor("wy0p", [128, 512], F16, kind="ExternalInput").ap()
    wy_d = nc.dram_tensor("wyp", [128, 512], F16, kind="ExternalInput").ap()
    wenh_d = nc.dram_tensor("wenhp", [128, 512], F16, kind="ExternalInput").ap()
    whh_d = nc.dram_tensor("whhp", [128, 512], F16, kind="ExternalInput").ap()
    wfeed_d = nc.dram_tensor("wfeedp", [128, 512], F16, kind="ExternalInput").ap()
    wgh_d = nc.dram_tensor("wghb", [128, 128], F16, kind="ExternalInput").ap()
    wi_d = nc.dram_tensor("wib", [128, 128], F16, kind="ExternalInput").ap()
    wf1_d = nc.dram_tensor("wf1b", [128, 128], F16, kind="ExternalInput").ap()
    wf2_d = nc.dram_tensor("wf2b", [128, 128], F16, kind="ExternalInput").ap()
    wf3_d = nc.dram_tensor("wf3c", [128, 4], F16, kind="ExternalInput").ap()
    b1s_d = nc.dram_tensor("b1s", [128, 12], F32, kind="ExternalInput").ap()
    b2_d = nc.dram_tensor("b2", [128, 1], F32, kind="ExternalInput").ap()
    bf34_d = nc.dram_tensor("bf34", [4, 1], F32, kind="ExternalInput").ap()
    out_d = nc.dram_tensor("out", [12, BC], F32, kind="ExternalOutput").ap()

    with tile.TileContext(nc) as tc:
        with tc.tile_pool(name="const", bufs=1) as cp, \
             tc.tile_pool(name="state", bufs=1) as st:

            def cload(shape, dt, src, tag):
                # weight/constant loads ride the idle SWDGE queue so the
                # sync/scalar HWDGE queues are free for the encoder stream
                t = cp.tile(shape, dt, tag=tag, name=tag)
                nc.gpsimd.dma_start(out=t, in_=src)
                return t

            identr = cload([128, 128], F32R, idr_d, "identr")
            wkv = cload([61, 60], F16, wkv_d, "wkv")
            ybd = cload([128, 6144], F16, ybd_d, "ybd")
            wy0 = cload([128, 512], F16, wy0_d, "wy0")
            wy = cload([128, 512], F16, wy_d, "wy")
            wenh = cload([128, 512], F16, wenh_d, "wenh")
            whh = cload([128, 512], F16, whh_d, "whh")
            wfeed = cload([128, 512], F16, wfeed_d, "wfeed")
            wgh = cload([128, 128], F16, wgh_d, "wgh")
            wib = cload([128, 128], F16, wi_d, "wib")
            wf1 = cload([128, 128], F16, wf1_d, "wf1")
            wf2 = cload([128, 128], F16, wf2_d, "wf2")
            wf3 = cload([128, 4], F16, wf3_d, "wf3")
            b1s = cload([128, 12], F32, b1s_d, "b1s")
            b2 = cload([128, 1], F32, b2_d, "b2")
            bf34 = cload([4, 1], F32, bf34_d, "bf34")
            zeros = st.tile([128, 512], F16, tag="zeros", name="zeros")
            nc.vector.memset(zeros, 0.0)
            ident = cp.tile([128, 128], F32, tag="ident", name="ident")
            make_identity(nc, ident)

            # ---- state tiles (allocated early; phase A fills per half) ----
            V4 = st.tile([128, 512], F16, tag="V4", name="V4")
            Vg4 = st.tile([128, 512], F16, tag="Vg4", name="Vg4")
            h4 = st.tile([128, 512], F16, tag="h4", name="h4")
            nc.sync.dma_start(out=h4, in_=h0_d)
            c4 = st.tile([128, 512], F16, tag="c4", name="c4")
            nc.vector.memset(c4, 0.0)
            gate4 = st.tile([128, 512], F16, tag="gate4", name="gate4")
            d4 = st.tile([128, 512], F16, tag="d4", name="d4")
            gh4 = st.tile([128, 512], F16, tag="gh4", name="gh4")
            z2t = st.tile([128, 512], F16, tag="z2t", name="z2t")
            out_store = st.tile([4, 6144], F32, tag="out_store", name="out_store")
            accS = st.tile([128, 960], F32, tag="accS", name="accS")

            with tc.tile_pool(name="ps", bufs=1, space="PSUM") as ps, \
                 tc.tile_pool(name="wk", bufs=2) as wk, \
                 tc.tile_pool(name="wk0", bufs=2) as wk0:
                ifo = ps.tile([128, 1536], F32, tag="ifo", name="ifo")
                gg = ps.tile([128, 512], F32, tag="gg", name="gg")
                gate_ps = ps.tile([128, 512], F32, tag="gate_ps", name="gate_ps")
                z1_ps = ps.tile([128, 512], F32, tag="z1_ps", name="z1_ps")
                z2_ps = ps.tile([128, 512], F32, tag="z2_ps", name="z2_ps")
                out_ps = ps.tile([128, 512], F32, tag="out_ps", name="out_ps")
                # dense f32 matmul block at t=0: trips the PE HAM to 8/8
                # before the encoder stream arrives, so the f32r accumulate
                # keeps pace with both DMA queues
                for _w in range(14):
                    nc.tensor.matmul(gate_ps[:, 0:128], ident, ident,
                                     start=True, stop=True)
                # early memsets: zero junk rows everywhere a full-AP read
                # happens; gg doubles as vps, out_ps as vgps (byte-disjoint
                # halves keep h0/h1 independent)
                nc.vector.memset(gg, 0.0)
                nc.vector.memset(out_ps, 0.0)
                nc.vector.memset(gate_ps, 0.0)
                nc.vector.memset(ifo[:, 0:768], 0.0)
                nc.vector.memset(z1_ps, 0.0)
                nc.vector.memset(z2_ps[:, 0:256], 0.0)

                # ---- phase A per half: 48-layer reduction on PE (f32r),
                # V/Vg, gate(0)/d/gh — half h's recurrence stream can start
                # while half 1-h is still streaming from HBM ----
                with tc.tile_pool(name="ld", bufs=10) as ldp:
                    for h in range(2):
                        hs = slice(256 * h, 256 * h + 256)
                        accH = (z1_ps[:, 0:480] if h == 0
                                else ifo[:, 1024:1504])
                        for l in range(ENC_LEN):
                            b = ldp.tile([128, 480], F32R, tag="ldb",
                                         name="ldb")
                            eng = nc.sync if l % 2 == 0 else nc.scalar
                            eng.dma_start(
                                out=b,
                                in_=enc_d[l, 1024 * h:1024 * h + 1024]
                                .rearrange("(p x) f -> p (x f)", p=128))
                            nc.tensor.matmul(accH, identr, b,
                                             start=(l == 0),
                                             stop=(l == ENC_LEN - 1))
                        nc.scalar.copy(accS[:, 480 * h:480 * h + 480], accH)
                        for c in range(NB):
                            et = wk0.tile([61, 256], F16, tag="et", name="et")
                            nc.vector.memset(et, 1.0)
                            for jp in range(2):
                                j8 = 4 * jp + c
                                ptr = z2_ps[0:60, 256 + 128 * jp:
                                            384 + 128 * jp]
                                nc.tensor.transpose(
                                    ptr,
                                    accS[:, 480 * h + 60 * j8:
                                         480 * h + 60 * j8 + 60],
                                    ident)
                                nc.vector.tensor_copy(
                                    et[0:60, 128 * jp:128 * jp + 128], ptr)
                            nc.tensor.matmul(
                                gg[32 * c:32 * c + 30, hs], wkv[:, 0:30], et,
                                start=True, stop=True,
                                tile_position=(0, 32 * c))
                            nc.tensor.matmul(
                                out_ps[32 * c:32 * c + 30, hs],
                                wkv[:, 30:60], et,
                                start=True, stop=True,
                                tile_position=(0, 32 * c))
                        nc.vector.tensor_copy(V4[:, hs], gg[:, hs])
                        nc.scalar.copy(Vg4[:, hs], out_ps[:, hs])
                        # gate(0), d(0), gh(0) for half h
                        nc.tensor.matmul(gate_ps[:, hs], wgh, h4[:, hs],
                                         start=True, stop=False)
                        nc.tensor.matmul(gate_ps[:, hs], wib, Vg4[:, hs],
                                         start=False, stop=True)
                        nc.scalar.activation(out=gate4[:, hs],
                                             in_=gate_ps[:, hs],
                                             func=AF.Sigmoid)
                        nc.vector.tensor_sub(d4[:, hs], h4[:, hs], V4[:, hs])
                        nc.vector.tensor_mul(gh4[:, hs], gate4[:, hs],
                                             d4[:, hs])
                        if h == 1:
                            nc.vector.memset(ifo[:, 768:1536], 0.0)
                            nc.vector.memset(z2_ps[:, 256:512], 0.0)

                sfo = wk.tile([128, 1536], F16, tag="sfo", name="sfo")
                tg = wk.tile([128, 512], F16, tag="tg", name="tg")
                ca = wk.tile([128, 512], F16, tag="ca", name="ca")
                cb = wk.tile([128, 512], F16, tag="cb", name="cb")
                tc4 = wk.tile([128, 512], F16, tag="tc4", name="tc4")
                z1 = wk.tile([128, 512], F16, tag="z1", name="z1")
                for s in range(steps):
                    wyp = wy0 if s == 0 else wy
                    for h in range(2):
                        hs = slice(256 * h, 256 * h + 256)
                        yrhs = ybd[:, 512 * s + 256 * h:512 * s + 256 * h + 256]
                        for b in range(4):
                            # half-major ifo layout: (h, gate) at 768h+256b
                            dst = (gg[:, hs] if b == 3
                                   else ifo[:, 768 * h + 256 * b:
                                            768 * h + 256 * b + 256])
                            lsl = slice(128 * b, 128 * b + 128)
                            nc.tensor.matmul(dst, wyp[:, lsl], yrhs,
                                             start=True, stop=False)
                            nc.tensor.matmul(dst, wenh[:, lsl], V4[:, hs],
                                             start=False, stop=False)
                            if s > 0:
                                nc.tensor.matmul(dst, wfeed[:, lsl],
                                                 z2t[:, hs],
                                                 start=False, stop=False)
                            nc.tensor.matmul(dst, wenh[:, lsl], gh4[:, hs],
                                             start=False, stop=False)
                            nc.tensor.matmul(dst, whh[:, lsl], h4[:, hs],
                                             start=False, stop=True)
                        h3 = slice(768 * h, 768 * h + 768)
                        nc.scalar.activation(out=sfo[:, h3], in_=ifo[:, h3],
                                             func=AF.Sigmoid)
                        nc.scalar.activation(out=tg[:, hs], in_=gg[:, hs],
                                             func=AF.Tanh)
                        fsl = slice(768 * h + 256, 768 * h + 512)
                        isl = slice(768 * h, 768 * h + 256)
                        osl = slice(768 * h + 512, 768 * h + 768)
                        nc.vector.tensor_mul(ca[:, hs], sfo[:, fsl], c4[:, hs])
                        nc.vector.tensor_mul(cb[:, hs], sfo[:, isl], tg[:, hs])
                        nc.vector.tensor_add(c4[:, hs], ca[:, hs], cb[:, hs])
                        nc.scalar.activation(out=tc4[:, hs], in_=c4[:, hs],
                                             func=AF.Tanh)
                        nc.vector.tensor_mul(h4[:, hs], sfo[:, osl], tc4[:, hs])
                        # head
                        nc.tensor.matmul(z1_ps[:, hs], wf1, h4[:, hs],
                                         start=True, stop=True)
                        nc.vector.scalar_tensor_tensor(
                            out=z1[:, hs], in0=z1_ps[:, hs],
                            scalar=b1s[:, s:s + 1], in1=zeros[:, hs],
                            op0=OP.add, op1=OP.max)
                        nc.tensor.matmul(z2_ps[:, hs], wf2, z1[:, hs],
                                         start=True, stop=True)
                        nc.vector.scalar_tensor_tensor(
                            out=z2t[:, hs], in0=z2_ps[:, hs], scalar=b2,
                            in1=zeros[:, hs], op0=OP.add, op1=OP.max)
                        nc.tensor.matmul(out_ps[0:4, hs], wf3, z2t[:, hs],
                                         start=True, stop=True)
                        nc.scalar.activation(
                            out=out_store[0:4, 512 * s + 256 * h:
                                          512 * s + 256 * h + 256],
                            in_=out_ps[0:4, hs], func=AF.Identity, bias=bf34)
                        if s < 11:
                            # gate(s+1), d, gh off the next step's chain
                            nc.tensor.matmul(gate_ps[:, hs], wgh, h4[:, hs],
                                             start=True, stop=False)
                            nc.tensor.matmul(gate_ps[:, hs], wib, Vg4[:, hs],
                                             start=False, stop=True)
                            nc.scalar.activation(out=gate4[:, hs],
                                                 in_=gate_ps[:, hs],
                                                 func=AF.Sigmoid)
                            nc.vector.tensor_sub(d4[:, hs], h4[:, hs],
                                                 V4[:, hs])
                            nc.vector.tensor_mul(gh4[:, hs], gate4[:, hs],
                                                 d4[:, hs])

                for s in range(steps):
                    nc.sync.dma_start(
                        out=out_d[s],
                        in_=out_store[0:4, 512 * s:512 * s + 512])
                if steps == 0:
                    nc.vector.tensor_copy(out_store[0:4, 0:512], gh4[0:4, :])
                    nc.sync.dma_start(out=out_d[0],
                                      in_=out_store[0:4, 0:512])

    nc.compile()
    return nc


def _make_weights(inp):
    f16 = np.float16
    W_ih = np.asarray(inp['W_ih'], np.float32)
    W_hh = np.asarray(inp['W_hh'], np.float32)
    bsum = np.asarray(inp['b_ih'] + inp['b_hh'], np.float32)
    Wg = np.asarray(inp['Wg'], np.float32)
    Wv, bv = np.asarray(inp['Wv'], np.float32), np.asarray(inp['bv'], np.float32)
    bg = np.asarray(inp['bg'], np.float32)
    Wf1 = np.asarray(inp['Wf1'], np.float32)
    bf1 = np.asarray(inp['bf1'], np.float32)
    Wf2 = np.asarray(inp['Wf2'], np.float32)
    bf2 = np.asarray(inp['bf2'], np.float32)
    Wf3 = np.asarray(inp['Wf3'], np.float32)
    bf3 = float(np.asarray(inp['bf3']).reshape(-1)[0])
    Wgc = Wg[:, DEC:]

    W = {}
    wkv61 = np.zeros((61, 60), np.float32)
    wkv61[:60, 0:30] = (Wv / 48.0).T
    wkv61[:60, 30:60] = ((Wgc @ Wv) / 48.0).T
    wkv61[60, 0:30] = bv
    wkv61[60, 30:60] = Wgc @ bv + bg
    W['wkv61'] = wkv61.astype(f16)

    def bd_pack(blocks):
        # blocks: list of 4 arrays [R_b, 30] -> [128, 512] pack of
        # block-diagonal lhsT (per bank), blocks at (32c, 32c)
        out = np.zeros((128, 512), np.float32)
        for b in range(4):
            P = blocks[b]
            R = P.shape[0]
            for c in range(NB):
                out[32 * c:32 * c + R, 128 * b + 32 * c:128 * b + 32 * c + 30] = P
        return out.astype(f16)

    def rows(b):
        return slice(ROW_MAP[b], ROW_MAP[b] + 30)

    wy0_blocks, wy_blocks = [], []
    for b in range(4):
        P0 = np.zeros((5, 30), np.float32)
        P0[0:4] = W_ih[rows(b), 0:4].T
        P0[4] = bsum[rows(b)]
        wy0_blocks.append(P0)
        P1 = np.zeros((5, 30), np.float32)
        P1[0:3] = W_ih[rows(b), 0:3].T
        P1[4] = bsum[rows(b)] + W_ih[rows(b), 3] * bf3
        wy_blocks.append(P1)
    W['wy0p'] = bd_pack(wy0_blocks)
    W['wyp'] = bd_pack(wy_blocks)
    W['wenhp'] = bd_pack([W_ih[rows(b), 4:34].T for b in range(4)])
    W['whhp'] = bd_pack([W_hh[rows(b), :].T for b in range(4)])
    W['wfeedp'] = bd_pack([np.outer(Wf3[0], W_ih[rows(b), 3]) for b in range(4)])

    def bd_one(P, Rout=30):
        out = np.zeros((128, 128), np.float32)
        R = P.shape[0]
        for c in range(NB):
            out[32 * c:32 * c + R, 32 * c:32 * c + P.shape[1]] = P
        return out.astype(f16)

    W['wghb'] = bd_one(Wg[:, :DEC].T)
    W['wib'] = bd_one(np.eye(30, dtype=np.float32))
    W['wf1b'] = bd_one(Wf1[:, :30].T)
    W['wf2b'] = bd_one(Wf2.T)
    wf3c = np.zeros((128, 4), np.float32)
    for c in range(NB):
        wf3c[32 * c:32 * c + 15, c] = Wf3[0]
    W['wf3c'] = wf3c.astype(f16)

    b1s = np.zeros((128, 12), np.float32)
    for c in range(NB):
        for s in range(12):
            b1s[32 * c:32 * c + 30, s] = bf1 + Wf1[:, 30] * ((s + 1) / 12.0)
    W['b1s'] = b1s
    b2 = np.zeros((128, 1), np.float32)
    for c in range(NB):
        b2[32 * c:32 * c + 15, 0] = bf2
    W['b2'] = b2
    W['bf34'] = np.full((4, 1), bf3, np.float32)
    W['identr'] = np.eye(128, dtype=np.float32)
    return W


def _make_ybd(y_dev):
    # y_dev: [2048, 13, 4] float32 in device order
    ybd = np.zeros((128, 12, 512), np.float16)
    for c in range(NB):
        seg = y_dev[512 * c:512 * (c + 1)]          # [512, 13, 4]
        ybd[32 * c + 0:32 * c + 4, 0] = np.nan_to_num(seg[:, 0, 0:4]).T
        for s in range(1, 12):
            ybd[32 * c + 0:32 * c + 3, s] = seg[:, s, 1:4].T
        ybd[32 * c + 4, :, :] = 1.0
    return np.ascontiguousarray(ybd.reshape(128, 6144))


def kernel(**inputs):
    import sys
    if '/opt/trn_rl_repo' not in sys.path:
        sys.path.insert(0, '/opt/trn_rl_repo')
    from concourse.bass_utils import run_bass_kernel_spmd

    if 1 not in _PROG:
        _PROG[1] = _build_program(1)
    nc = _PROG[1]

    inputs = {k: np.asarray(v) for k, v in inputs.items()}
    W = _make_weights(inputs)
    lod = _local_of_dev()

    enc_full = np.asarray(inputs['encoder_outputs'], np.float32)
    y_full = np.asarray(inputs['y'], np.float32)
    hid_full = np.asarray(inputs['hidden'], np.float32)

    in_maps = []
    for core in range(NCORES):
        gsl = slice(core * BC, (core + 1) * BC)
        h_dev = hid_full[gsl][lod]                       # [2048, 30]
        h0 = np.zeros((128, 512), np.float16)
        for c in range(NB):
            h0[32 * c:32 * c + 30, :] = h_dev[512 * c:512 * (c + 1)].T
        m = dict(W)
        m['enc'] = np.ascontiguousarray(enc_full[:, gsl, :])
        m['ybd'] = _make_ybd(y_full[gsl][lod])
        m['h0'] = h0
        in_maps.append(m)

    res = run_bass_kernel_spmd(nc, in_maps, list(range(NCORES)))

    out = np.zeros((12, B), np.float32)
    for core in range(NCORES):
        oc = res.results[core]['out']                    # [12, 2048] dev order
        out[:, core * BC + lod] = oc
    return out


if __name__ == '__main__':
    pass
